# revision 1
# baseline (speedup 1.0000x reference)
"""CrossAttention2d Trainium2 kernel.

Data-parallel over batch: 16 batches / 8 cores = 2 per core. Weights
replicated; no collectives. Heavy matmuls in bf16 with fp32 PSUM
accumulation; layernorm statistics in fp32.

Weight layout prep (transpose for the TensorE stationary slot, bf16
cast, gamma fold, LN rank-1 row sums) happens on host in kernel().

Math notes (per batch):
  x:[C,HW] channel-LN folded into the q projection:
    nd = g*(x-mu)*rs + b  (mu,rs per spatial column p)
    q  = Wq@nd + bq = rs_p * [ (Wq*g)@x  +  wqgsum*(-mu)^T + (wqb+bq)*sd^T ]
  with sd = 1/rs, wqgsum[o] = sum_c (Wq*g)[o,c], wqb[o] = sum_c Wq[o,c]*b[c].
  The rank-1 terms are two K=1 matmuls accumulated into the same PSUM
  group as the projection. Same fold for the encoder LN into kv.
  Attention computed as S^T[t,p] per head so the padding mask is a
  per-partition bias of exp() and the softmax sum is a ones-matmul.
  exp(S*0.125 - 1e4*pad) needs no max-subtraction (|S*0.125| < ~10).
"""

import ml_dtypes
import numpy as np

import concourse.bass as bass
import concourse.bacc as bacc
import concourse.mybir as mybir
import concourse.tile as tile
from concourse.masks import make_identity
from concourse.bass_utils import run_bass_kernel_spmd

F32 = mybir.dt.float32
BF16 = mybir.dt.bfloat16
I32 = mybir.dt.int32
BF = ml_dtypes.bfloat16

B, C, HW, S, E, H, D = 16, 1024, 1024, 256, 768, 16, 64
NCORES = 8
BPC = B // NCORES  # batches per core
EPS = 1e-5
CI = C // 128      # 8 c-tiles
EI = E // 128      # 6 e-tiles
JI = 2 * C // 128  # 16 kv row-tiles

_CACHE = {}


def _build(nc: bass.Bass):
    xd = nc.dram_tensor("x", [BPC, C, HW], F32, kind="ExternalInput")[:, :, :]
    encd = nc.dram_tensor("enc", [BPC, S, E], F32, kind="ExternalInput")[:, :, :]
    padd = nc.dram_tensor("padding", [BPC, S], I32, kind="ExternalInput")[:, :]
    wqTd = nc.dram_tensor("wqT", [128, CI, C], BF16, kind="ExternalInput")[:, :, :]
    wkvTd = nc.dram_tensor("wkvT", [128, EI, 2 * C], BF16, kind="ExternalInput")[:, :, :]
    woTd = nc.dram_tensor("woT", [128, CI, C], BF16, kind="ExternalInput")[:, :, :]
    wqrd = nc.dram_tensor("wqr", [2, C], BF16, kind="ExternalInput")[:, :]
    wkvrd = nc.dram_tensor("wkvr", [2, 2 * C], BF16, kind="ExternalInput")[:, :]
    bod = nc.dram_tensor("bo", [C], F32, kind="ExternalInput")[:]
    outd = nc.dram_tensor("out", [BPC, C, HW], F32, kind="ExternalOutput")[:, :, :]

    with tile.TileContext(nc) as tc:
        con = tc.alloc_tile_pool(name="con", bufs=1)
        wgt = tc.alloc_tile_pool(name="wgt", bufs=1)
        scr = tc.alloc_tile_pool(name="scr", bufs=1, space="PSUM")
        scrt = scr.tile([1, 512], F32)

        def observe(t):
            # A transpose is one PE instruction with a single sync-wait slot,
            # so it cannot wait on both its input DMA and the identity /
            # PSUM-slot release. A regular matmul (LDWEIGHTS+MATMUL pair) has
            # two slots; a throwaway one makes PE observe the fresh DMA.
            nc.tensor.matmul(scrt, t[:, 0:1], t[:, 0:512], start=True, stop=True)

        ones_cb = con.tile([128, 1], BF16)
        nc.vector.memset(ones_cb, 1.0)
        ones_cf = con.tile([128, 1], F32)
        nc.vector.memset(ones_cf, 1.0)
        ones1 = con.tile([1, 128], F32)
        nc.vector.memset(ones1, 1.0)
        ones1b = con.tile([1, 128], BF16)
        nc.vector.memset(ones1b, 1.0)
        idf = con.tile([128, 128], F32)
        make_identity(nc, idf)
        idb = con.tile([128, 128], BF16)
        make_identity(nc, idb)
        eps11 = con.tile([1, 1], F32)
        nc.vector.memset(eps11, EPS)
        junk = con.tile([1, 8], F32)  # target for DMA-observation touches

        bo_col = con.tile([128, CI], F32)
        nc.sync.dma_start(out=bo_col, in_=bod.rearrange("(a p) -> p a", p=128))

        # persistent weights (pre-transposed on host)
        wqT = wgt.tile([128, CI, C], BF16)
        nc.sync.dma_start(out=wqT, in_=wqTd)
        wkvT = wgt.tile([128, EI, 2 * C], BF16)
        nc.sync.dma_start(out=wkvT, in_=wkvTd)
        woT = wgt.tile([128, CI, C], BF16)
        nc.sync.dma_start(out=woT, in_=woTd)
        wqr = wgt.tile([2, C], BF16)      # [wqgsum; wqb+bq]
        nc.sync.dma_start(out=wqr, in_=wqrd)
        wkvr = wgt.tile([2, 2 * C], BF16)
        nc.sync.dma_start(out=wkvr, in_=wkvrd)

        # ---------------- per-batch ----------------
        per = tc.alloc_tile_pool(name="per", bufs=1)
        for b in range(BPC):
            sfx = f"_b{b}"
            xsb = per.tile([128, CI, HW], BF16, tag="xsb")
            qsb = per.tile([128, CI, HW], BF16, tag="qsb")
            kvT = per.tile([128, JI, S], BF16, tag="kvT")   # [j%128, ji, t]
            vnat = per.tile([128, 2, C], BF16, tag="vnat")  # [t%128, si, j']
            eTf = per.tile([128, EI, S], F32, tag="eTf")
            eTb = per.tile([128, EI, S], BF16, tag="eTb")
            a_sb = per.tile([128, HW], F32, tag="a_sb")     # rs broadcast
            a2_sb = per.tile([128, S], F32, tag="a2_sb")
            nmu_x = per.tile([1, HW], BF16, tag="nmu_x")   # -mu
            sd_x = per.tile([1, HW], BF16, tag="sd_x")     # sqrt(var+eps)
            sdf_x = per.tile([1, HW], F32, tag="sdf_x")
            r1x = per.tile([2, HW], BF16, tag="r1x")       # [-mu; sd] packed
            nmu_e = per.tile([1, S], BF16, tag="nmu_e")
            sd_e = per.tile([1, S], BF16, tag="sd_e")
            sdf_e = per.tile([1, S], F32, tag="sdf_e")
            r1e = per.tile([2, S], BF16, tag="r1e")
            padneg = per.tile([128, 2], F32, tag="padneg")

            # ---- encoder: load, transpose, LN stats ----
            with tc.tile_pool(name="enc_sb" + sfx, bufs=1) as esbp, \
                 tc.tile_pool(name="enc_sq" + sfx, bufs=2) as esqp, \
                 tc.tile_pool(name="enc_tp" + sfx, bufs=2, space="PSUM") as etp, \
                 tc.tile_pool(name="enc_row" + sfx, bufs=2, space="PSUM") as erow, \
                 tc.tile_pool(name="enc_a2" + sfx, bufs=1, space="PSUM") as ea2, \
                 tc.tile_pool(name="rows" + sfx, bufs=3) as rows:

                esb = esbp.tile([128, 2, E], F32)
                nc.sync.dma_start(out=esb, in_=encd[b].rearrange("(si p) e -> p si e", p=128))
                observe(esb[:, 0, :])
                for si in range(2):
                    for ei in range(EI):
                        tp = etp.tile([128, 128], F32, tag="etp")
                        nc.tensor.transpose(tp, esb[:, si, ei * 128:(ei + 1) * 128], idf)
                        nc.vector.tensor_copy(out=eTf[:, ei, si * 128:(si + 1) * 128], in_=tp)
                nc.gpsimd.tensor_copy(out=eTb, in_=eTf)

                sume = erow.tile([1, S], F32, tag="erow")
                sqe = erow.tile([1, S], F32, tag="erow")
                for ei in range(EI):
                    esq = esqp.tile([128, S], BF16, tag="esq")
                    nc.vector.tensor_mul(esq, eTb[:, ei, :], eTb[:, ei, :])
                    nc.tensor.matmul(sume, ones_cb, eTb[:, ei, :],
                                     start=(ei == 0), stop=(ei == EI - 1))
                    nc.tensor.matmul(sqe, ones_cb, esq,
                                     start=(ei == 0), stop=(ei == EI - 1))
                nc.scalar.activation(out=nmu_e, in_=sume,
                                     func=mybir.ActivationFunctionType.Copy,
                                     scale=-1.0 / E)
                mu2 = rows.tile([1, S], F32, tag="r_s")
                nc.scalar.activation(out=mu2, in_=nmu_e,
                                     func=mybir.ActivationFunctionType.Square)
                ex2 = rows.tile([1, S], F32, tag="r_s")
                nc.scalar.activation(out=ex2, in_=sqe,
                                     func=mybir.ActivationFunctionType.Copy,
                                     scale=1.0 / E)
                var = rows.tile([1, S], F32, tag="r_s")
                nc.vector.tensor_tensor(out=var, in0=ex2, in1=mu2,
                                        op=mybir.AluOpType.subtract)
                nc.scalar.activation(out=sd_e, in_=var,
                                     func=mybir.ActivationFunctionType.Sqrt,
                                     bias=eps11)
                nc.scalar.activation(out=sdf_e, in_=var,
                                     func=mybir.ActivationFunctionType.Sqrt,
                                     bias=eps11)
                rs2 = rows.tile([1, S], F32, tag="r_s")
                nc.vector.reciprocal(out=rs2, in_=sdf_e)
                nc.sync.dma_start(out=r1e[0:1, :], in_=nmu_e)
                nc.sync.dma_start(out=r1e[1:2, :], in_=sd_e)
                a2ps = ea2.tile([128, S], F32)
                nc.tensor.matmul(a2ps, ones1, rs2, start=True, stop=True)
                nc.vector.tensor_copy(out=a2_sb, in_=a2ps)

            with tc.tile_pool(name="kv_ps" + sfx, bufs=2, space="PSUM") as kvp:
                for ji in range(JI):
                    kvps = kvp.tile([128, S], F32)
                    for ei in range(EI):
                        nc.tensor.matmul(kvps, wkvT[:, ei, ji * 128:(ji + 1) * 128],
                                         eTb[:, ei, :],
                                         start=(ei == 0), stop=False)
                    nc.tensor.matmul(kvps, wkvr[:, ji * 128:(ji + 1) * 128],
                                     r1e, start=False, stop=True)
                    nc.vector.tensor_mul(kvT[:, ji, :], kvps, a2_sb)

            with tc.tile_pool(name="v_tp" + sfx, bufs=2, space="PSUM") as vtp:
                for jj in range(CI):
                    for si in range(2):
                        tp = vtp.tile([128, 128], BF16, tag="vtp")
                        nc.tensor.transpose(tp, kvT[:, CI + jj, si * 128:(si + 1) * 128], idb)
                        nc.vector.tensor_copy(out=vnat[:, si, jj * 128:(jj + 1) * 128], in_=tp)

            # ---- decoder x: load, stats, q ----
            with tc.tile_pool(name="x_f32" + sfx, bufs=3) as xfp, \
                 tc.tile_pool(name="x_sq" + sfx, bufs=2) as xsqp, \
                 tc.tile_pool(name="x_row" + sfx, bufs=2, space="PSUM") as xrow, \
                 tc.tile_pool(name="x_a" + sfx, bufs=1, space="PSUM") as xa, \
                 tc.tile_pool(name="rows2" + sfx, bufs=3) as rows:

                sumx = xrow.tile([1, HW], F32, tag="xrow")
                sqx = xrow.tile([1, HW], F32, tag="xrow")
                for ci in range(CI):
                    xf = xfp.tile([128, HW], F32, tag="xf")
                    nc.sync.dma_start(
                        out=xf, in_=xd[b].rearrange("(ci p) hw -> p ci hw", p=128)[:, ci, :])
                    nc.gpsimd.tensor_copy(out=xsb[:, ci, :], in_=xf)
                    xq = xsqp.tile([128, HW], BF16, tag="xq")
                    nc.vector.tensor_mul(xq, xsb[:, ci, :], xsb[:, ci, :])
                    for ch in range(2):
                        sl = slice(ch * 512, (ch + 1) * 512)
                        nc.tensor.matmul(sumx[:, sl], ones_cb, xsb[:, ci, sl],
                                         start=(ci == 0), stop=(ci == CI - 1))
                        nc.tensor.matmul(sqx[:, sl], ones_cb, xq[:, sl],
                                         start=(ci == 0), stop=(ci == CI - 1))
                nc.scalar.activation(out=nmu_x, in_=sumx,
                                     func=mybir.ActivationFunctionType.Copy,
                                     scale=-1.0 / C)
                mu2 = rows.tile([1, HW], F32, tag="r_hw")
                nc.scalar.activation(out=mu2, in_=nmu_x,
                                     func=mybir.ActivationFunctionType.Square)
                ex2 = rows.tile([1, HW], F32, tag="r_hw")
                nc.scalar.activation(out=ex2, in_=sqx,
                                     func=mybir.ActivationFunctionType.Copy,
                                     scale=1.0 / C)
                var = rows.tile([1, HW], F32, tag="r_hw")
                nc.vector.tensor_tensor(out=var, in0=ex2, in1=mu2,
                                        op=mybir.AluOpType.subtract)
                nc.scalar.activation(out=sd_x, in_=var,
                                     func=mybir.ActivationFunctionType.Sqrt,
                                     bias=eps11)
                nc.scalar.activation(out=sdf_x, in_=var,
                                     func=mybir.ActivationFunctionType.Sqrt,
                                     bias=eps11)
                rsx = rows.tile([1, HW], F32, tag="r_hw")
                nc.vector.reciprocal(out=rsx, in_=sdf_x)
                nc.sync.dma_start(out=r1x[0:1, :], in_=nmu_x)
                nc.sync.dma_start(out=r1x[1:2, :], in_=sd_x)
                aps = xa.tile([128, HW], F32)
                for ch in range(2):
                    sl = slice(ch * 512, (ch + 1) * 512)
                    nc.tensor.matmul(aps[:, sl], ones1, rsx[:, sl], start=True, stop=True)
                nc.vector.tensor_copy(out=a_sb, in_=aps)

            with tc.tile_pool(name="q_ps" + sfx, bufs=2, space="PSUM") as qpp:
                for oi in range(CI):
                    qps = qpp.tile([128, HW], F32)
                    for ch in range(2):
                        sl = slice(ch * 512, (ch + 1) * 512)
                        for ci in range(CI):
                            nc.tensor.matmul(qps[:, sl],
                                             wqT[:, ci, oi * 128:(oi + 1) * 128],
                                             xsb[:, ci, sl],
                                             start=(ci == 0), stop=False)
                        nc.tensor.matmul(qps[:, sl],
                                         wqr[:, oi * 128:(oi + 1) * 128],
                                         r1x[:, sl], start=False, stop=True)
                    nc.vector.tensor_mul(qsb[:, oi, :], qps, a_sb)

            # ---- padding bias ----
            with tc.tile_pool(name="pad" + sfx, bufs=1) as padp:
                padi = padp.tile([128, 2], I32)
                nc.sync.dma_start(out=padi, in_=padd[b].rearrange("(si p) -> p si", p=128))
                padf = padp.tile([128, 2], F32)
                nc.vector.tensor_copy(out=padf, in_=padi)
                nc.scalar.mul(out=padneg, in_=padf, mul=-10000.0)

            # ---- attention ----
            ysb = per.tile([128, CI, HW], BF16, tag="ysb")
            with tc.tile_pool(name="s_ps" + sfx, bufs=2, space="PSUM") as spp, \
                 tc.tile_pool(name="z_ps" + sfx, bufs=1, space="PSUM") as zpp, \
                 tc.tile_pool(name="rb_ps" + sfx, bufs=1, space="PSUM") as rbp, \
                 tc.tile_pool(name="y_ps" + sfx, bufs=1, space="PSUM") as ypp, \
                 tc.tile_pool(name="att_sb" + sfx, bufs=3) as attp, \
                 tc.tile_pool(name="att_r" + sfx, bufs=2) as attr:
                for h in range(H):
                    ji = h // 2
                    dof = (h % 2) * 64
                    for pc in range(2):
                        psl = slice(pc * 512, (pc + 1) * 512)
                        sps = spp.tile([128, 2, 512], F32, tag="sps")
                        for si in range(2):
                            nc.tensor.matmul(
                                sps[:, si, :],
                                kvT[dof:dof + 64, ji, si * 128:(si + 1) * 128],
                                qsb[dof:dof + 64, ji, psl],
                                start=True, stop=True)
                        eb = attp.tile([128, 2, 512], BF16, tag="eb")
                        for si in range(2):
                            nc.scalar.activation(out=eb[:, si, :], in_=sps[:, si, :],
                                                 func=mybir.ActivationFunctionType.Exp,
                                                 bias=padneg[:, si:si + 1],
                                                 scale=0.125)
                        zps = zpp.tile([1, 512], F32, tag="zps")
                        for si in range(2):
                            nc.tensor.matmul(zps, ones_cb, eb[:, si, :],
                                             start=(si == 0), stop=(si == 1))
                        zrow = attr.tile([1, 512], BF16, tag="zrow")
                        nc.scalar.copy(out=zrow, in_=zps)
                        zbps = rbp.tile([64, 512], F32, tag="zbps")
                        nc.tensor.matmul(zbps, ones1b[0:1, 0:64], zrow,
                                         start=True, stop=True)
                        rbsb = attr.tile([64, 512], F32, tag="rbsb")
                        nc.vector.reciprocal(out=rbsb, in_=zbps)
                        yps = ypp.tile([64, 512], F32, tag="yps")
                        for si in range(2):
                            nc.tensor.matmul(yps, vnat[:, si, h * 64:(h + 1) * 64],
                                             eb[:, si, :],
                                             start=(si == 0), stop=(si == 1))
                        nc.vector.tensor_mul(ysb[dof:dof + 64, ji, psl], yps, rbsb)

            # ---- output: Wo @ y + bo + x ----
            with tc.tile_pool(name="o_ps" + sfx, bufs=2, space="PSUM") as opp, \
                 tc.tile_pool(name="o_sb" + sfx, bufs=2) as osp, \
                 tc.tile_pool(name="xr_sb" + sfx, bufs=2) as xrp:
                for oi in range(CI):
                    ops = opp.tile([128, HW], F32)
                    for ch in range(2):
                        sl = slice(ch * 512, (ch + 1) * 512)
                        for ci in range(CI):
                            nc.tensor.matmul(ops[:, sl],
                                             woT[:, ci, oi * 128:(oi + 1) * 128],
                                             ysb[:, ci, sl],
                                             start=(ci == 0), stop=(ci == CI - 1))

                    xres = xrp.tile([128, HW], F32, tag="xres")
                    nc.sync.dma_start(
                        out=xres, in_=xd[b].rearrange("(oi p) hw -> p oi hw", p=128)[:, oi, :])
                    # absorb the DMA wait on DVE so the residual add needs
                    # only the single PE wait (1 sync slot per DVE op)
                    nc.vector.tensor_copy(out=junk, in_=xres[0:1, 0:8])
                    osb = osp.tile([128, HW], F32, tag="osb")
                    nc.vector.scalar_tensor_tensor(
                        out=osb, in0=ops, scalar=bo_col[:, oi:oi + 1], in1=xres,
                        op0=mybir.AluOpType.add, op1=mybir.AluOpType.add)
                    nc.sync.dma_start(
                        out=outd[b].rearrange("(oi p) hw -> p oi hw", p=128)[:, oi, :],
                        in_=osb)
        per.release()
        scr.release()
        wgt.release()
        con.release()
    return nc


def _get_nc():
    if "nc" not in _CACHE:
        nc = bacc.Bacc()
        _build(nc)
        nc.compile()
        _CACHE["nc"] = nc
    return _CACHE["nc"]


def _prep_weights(gamma_dec, beta_dec, gamma_enc, beta_enc, Wq, bq, Wkv, bkv, Wo, bo):
    Wq = np.asarray(Wq, np.float32)
    Wkv = np.asarray(Wkv, np.float32)
    Wo = np.asarray(Wo, np.float32)
    gd = np.asarray(gamma_dec, np.float32)
    bd = np.asarray(beta_dec, np.float32)
    ge = np.asarray(gamma_enc, np.float32)
    be = np.asarray(beta_enc, np.float32)

    def packT(w):  # [o, c] -> [128, c//128, o] bf16 (stationary layout)
        o, c = w.shape
        t = np.ascontiguousarray(w.T.reshape(c // 128, 128, o).transpose(1, 0, 2))
        return t.astype(BF)

    wqg_full = Wq * gd[None, :]
    wqT = packT(wqg_full)
    wkvg_full = Wkv * ge[None, :]
    wkvT = packT(wkvg_full)
    woT = packT(Wo)
    # row sums from the bf16-rounded weights to match the device matmuls
    wqg = wqg_full.astype(BF).astype(np.float32).sum(axis=1)         # [C]
    wqb = Wq.astype(BF).astype(np.float32) @ bd + np.asarray(bq, np.float32)
    wkvg = wkvg_full.astype(BF).astype(np.float32).sum(axis=1)
    wkvb = Wkv.astype(BF).astype(np.float32) @ be + np.asarray(bkv, np.float32)
    wqr = np.ascontiguousarray(np.stack([wqg, wqb]).astype(BF))      # [2, C]
    wkvr = np.ascontiguousarray(np.stack([wkvg, wkvb]).astype(BF))
    return dict(
        wqT=wqT, wkvT=wkvT, woT=woT, wqr=wqr, wkvr=wkvr,
        bo=np.asarray(bo, np.float32),
    )


def kernel(x, enc, padding, gamma_dec, beta_dec, gamma_enc, beta_enc,
           Wq, bq, Wkv, bkv, Wo, bo, _trace=False):
    nc = _get_nc()
    x = np.ascontiguousarray(np.asarray(x, np.float32)).reshape(B, C, HW)
    enc = np.ascontiguousarray(np.asarray(enc, np.float32))
    padding = np.ascontiguousarray(np.asarray(padding, np.int32))
    wdict = _prep_weights(gamma_dec, beta_dec, gamma_enc, beta_enc,
                          Wq, bq, Wkv, bkv, Wo, bo)
    in_maps = []
    for c in range(NCORES):
        m = dict(wdict)
        m["x"] = np.ascontiguousarray(x[c * BPC:(c + 1) * BPC])
        m["enc"] = np.ascontiguousarray(enc[c * BPC:(c + 1) * BPC])
        m["padding"] = np.ascontiguousarray(padding[c * BPC:(c + 1) * BPC])
        in_maps.append(m)
    res = run_bass_kernel_spmd(nc, in_maps, core_ids=list(range(NCORES)),
                               trace=_trace)
    if _trace:
        _CACHE["last_results"] = res
    out = np.concatenate([res.results[c]["out"] for c in range(NCORES)], axis=0)
    return out.reshape(B, C, 32, 32).astype(np.float32)



# revision 12
# speedup vs baseline: 1.3113x; 1.3113x over previous
"""CrossAttention2d Trainium2 kernel.

Data-parallel over batch: 16 batches / 8 cores = 2 per core. Weights
replicated; no collectives. Heavy matmuls in bf16 with fp32 PSUM
accumulation; layernorm statistics in fp32.

Weight layout prep (transpose for the TensorE stationary slot, bf16
cast, gamma fold, LN rank-1 row sums) happens on host in kernel().

Math notes (per batch):
  x:[C,HW] channel-LN folded into the q projection:
    nd = g*(x-mu)*rs + b  (mu,rs per spatial column p)
    q  = Wq@nd + bq = rs_p * [ (Wq*g)@x  +  wqgsum*(-mu)^T + (wqb+bq)*sd^T ]
  with sd = 1/rs, wqgsum[o] = sum_c (Wq*g)[o,c], wqb[o] = sum_c Wq[o,c]*b[c].
  The rank-1 terms are two K=1 matmuls accumulated into the same PSUM
  group as the projection. Same fold for the encoder LN into kv.
  Attention computed as S^T[t,p] per head so the padding mask is a
  per-partition bias of exp() and the softmax sum is a ones-matmul.
  exp(S*0.125 - 1e4*pad) needs no max-subtraction (|S*0.125| < ~10).
"""

import ml_dtypes
import numpy as np

import concourse.bass as bass
import concourse.bacc as bacc
import concourse.mybir as mybir
import concourse.tile as tile
from concourse.masks import make_identity
from concourse.bass_utils import run_bass_kernel_spmd

F32 = mybir.dt.float32
BF16 = mybir.dt.bfloat16
I32 = mybir.dt.int32
BF = ml_dtypes.bfloat16

B, C, HW, S, E, H, D = 16, 1024, 1024, 256, 768, 16, 64
NCORES = 8
BPC = B // NCORES  # batches per core
EPS = 1e-5
CI = C // 128      # 8 c-tiles
EI = E // 128      # 6 e-tiles
JI = 2 * C // 128  # 16 kv row-tiles

_CACHE = {}


def _build(nc: bass.Bass):
    xd = nc.dram_tensor("x", [BPC, C, HW], F32, kind="ExternalInput")[:, :, :]
    encd = nc.dram_tensor("enc", [BPC, S, E], F32, kind="ExternalInput")[:, :, :]
    padd = nc.dram_tensor("padding", [BPC, S], I32, kind="ExternalInput")[:, :]
    wqTd = nc.dram_tensor("wqT", [128, CI, C], BF16, kind="ExternalInput")[:, :, :]
    wkvTd = nc.dram_tensor("wkvT", [128, EI, 2 * C], BF16, kind="ExternalInput")[:, :, :]
    woTd = nc.dram_tensor("woT", [128, CI, C], BF16, kind="ExternalInput")[:, :, :]
    wqrd = nc.dram_tensor("wqr", [2, C], BF16, kind="ExternalInput")[:, :]
    wkvrd = nc.dram_tensor("wkvr", [2, 2 * C], BF16, kind="ExternalInput")[:, :]
    bod = nc.dram_tensor("bo", [C], F32, kind="ExternalInput")[:]
    pmd = nc.dram_tensor("pmask", [2, 128], BF16, kind="ExternalInput")[:, :]
    outd = nc.dram_tensor("out", [BPC, C, HW], F32, kind="ExternalOutput")[:, :, :]

    with tile.TileContext(nc) as tc:
        con = tc.alloc_tile_pool(name="con", bufs=1)
        wgt = tc.alloc_tile_pool(name="wgt", bufs=1)
        scr = tc.alloc_tile_pool(name="scr", bufs=1, space="PSUM")
        scrt = scr.tile([1, 512], F32)

        def observe(t):
            # A transpose is one PE instruction with a single sync-wait slot,
            # so it cannot wait on both its input DMA and the identity /
            # PSUM-slot release. A regular matmul (LDWEIGHTS+MATMUL pair) has
            # two slots; a throwaway one makes PE observe the fresh DMA.
            nc.tensor.matmul(scrt, t[:, 0:1], t[:, 0:512], start=True, stop=True)

        ones_cb = con.tile([128, 1], BF16)
        nc.vector.memset(ones_cb, 1.0)
        ones_cf = con.tile([128, 1], F32)
        nc.vector.memset(ones_cf, 1.0)
        ones1 = con.tile([1, 128], F32)
        nc.vector.memset(ones1, 1.0)
        ones1b = con.tile([1, 128], BF16)
        nc.vector.memset(ones1b, 1.0)
        idf = con.tile([128, 128], F32)
        make_identity(nc, idf)
        idb = con.tile([128, 128], BF16)
        make_identity(nc, idb)
        eps11 = con.tile([1, 1], F32)
        nc.vector.memset(eps11, EPS)
        # head-pair broadcast mask (host-built): out rows 0-63 <- zinv row 0
        # (even head), rows 64-127 <- zinv row 1 (odd head)
        pmask = con.tile([2, 128], BF16)
        nc.sync.dma_start(out=pmask, in_=pmd)
        junk = con.tile([1, 8], F32)  # target for DMA-observation touches

        bo_col = con.tile([128, CI], F32)
        nc.sync.dma_start(out=bo_col, in_=bod.rearrange("(a p) -> p a", p=128))

        # persistent weights (pre-transposed on host)
        wqT = wgt.tile([128, CI, C], BF16)
        nc.sync.dma_start(out=wqT, in_=wqTd)
        wkvT = wgt.tile([128, EI, 2 * C], BF16)
        nc.sync.dma_start(out=wkvT, in_=wkvTd)
        woT = wgt.tile([128, CI, C], BF16)
        nc.sync.dma_start(out=woT, in_=woTd)
        wqr = wgt.tile([2, C], BF16)      # [wqgsum; wqb+bq]
        nc.sync.dma_start(out=wqr, in_=wqrd)
        wkvr = wgt.tile([2, 2 * C], BF16)
        nc.sync.dma_start(out=wkvr, in_=wkvrd)

        # ---------------- per-batch ----------------
        per = tc.alloc_tile_pool(name="per", bufs=1)
        for b in range(BPC):
            sfx = f"_b{b}"
            xsb = per.tile([128, CI, HW], BF16, tag="xsb")
            qsb = per.tile([128, CI, HW], BF16, tag="qsb")
            kvT = per.tile([128, JI, S], BF16, tag="kvT")   # [j%128, ji, t]
            vnat = per.tile([128, 2, C], BF16, tag="vnat")  # [t%128, si, j']
            eTb = per.tile([128, EI, S], BF16, tag="eTb")
            a_sb = per.tile([128, HW], F32, tag="a_sb")     # rs broadcast
            a2_sb = per.tile([128, S], F32, tag="a2_sb")
            nmu_x = per.tile([1, HW], BF16, tag="nmu_x")   # -mu
            sd_x = per.tile([1, HW], BF16, tag="sd_x")     # sqrt(var+eps)
            r1x = per.tile([2, HW], BF16, tag="r1x")       # [-mu; sd] packed
            nmu_e = per.tile([1, S], BF16, tag="nmu_e")
            sd_e = per.tile([1, S], BF16, tag="sd_e")
            r1e = per.tile([2, S], BF16, tag="r1e")
            padneg = per.tile([128, 2], F32, tag="padneg")

            # ---- encoder: load, transpose, LN stats ----
            with tc.tile_pool(name="enc_sb" + sfx, bufs=1) as esbp, \
                 tc.tile_pool(name="enc_sq" + sfx, bufs=2) as esqp, \
                 tc.tile_pool(name="enc_tp" + sfx, bufs=2, space="PSUM") as etp, \
                 tc.tile_pool(name="enc_row" + sfx, bufs=2, space="PSUM") as erow, \
                 tc.tile_pool(name="enc_a2" + sfx, bufs=1, space="PSUM") as ea2, \
                 tc.tile_pool(name="rows" + sfx, bufs=3) as rows:

                esb = esbp.tile([128, 2, E], F32)
                nc.sync.dma_start(out=esb, in_=encd[b].rearrange("(si p) e -> p si e", p=128))
                observe(esb[:, 0, :])
                for si in range(2):
                    for ei in range(EI):
                        tp = etp.tile([128, 128], F32, tag="etp")
                        nc.tensor.transpose(tp, esb[:, si, ei * 128:(ei + 1) * 128], idf)
                        nc.vector.tensor_copy(out=eTb[:, ei, si * 128:(si + 1) * 128], in_=tp)

                sume = erow.tile([1, S], F32, tag="erow")
                sqe = erow.tile([1, S], F32, tag="erow")
                for ei in range(EI):
                    esq = esqp.tile([128, S], BF16, tag="esq")
                    nc.vector.tensor_mul(esq, eTb[:, ei, :], eTb[:, ei, :])
                    nc.tensor.matmul(sume, ones_cb, eTb[:, ei, :],
                                     start=(ei == 0), stop=(ei == EI - 1))
                    nc.tensor.matmul(sqe, ones_cb, esq,
                                     start=(ei == 0), stop=(ei == EI - 1))
                nc.scalar.activation(out=nmu_e, in_=sume,
                                     func=mybir.ActivationFunctionType.Copy,
                                     scale=-1.0 / E)
                mu2 = rows.tile([1, S], F32, tag="r_s")
                nc.scalar.activation(out=mu2, in_=nmu_e,
                                     func=mybir.ActivationFunctionType.Square)
                ex2 = rows.tile([1, S], F32, tag="r_s")
                nc.scalar.activation(out=ex2, in_=sqe,
                                     func=mybir.ActivationFunctionType.Copy,
                                     scale=1.0 / E)
                var = rows.tile([1, S], F32, tag="r_s")
                nc.vector.tensor_tensor(out=var, in0=ex2, in1=mu2,
                                        op=mybir.AluOpType.subtract)
                # sd/rs via exp(+-0.5*ln(var+eps)): stays in the
                # natural_log_exp table set (no Sqrt set switch, no DVE
                # iterative reciprocal)
                lnv = rows.tile([1, S], F32, tag="r_s")
                nc.scalar.activation(out=lnv, in_=var,
                                     func=mybir.ActivationFunctionType.Ln,
                                     bias=eps11)
                nc.scalar.activation(out=sd_e, in_=lnv,
                                     func=mybir.ActivationFunctionType.Exp,
                                     scale=0.5)
                rs2 = rows.tile([1, S], F32, tag="r_s")
                nc.scalar.activation(out=rs2, in_=lnv,
                                     func=mybir.ActivationFunctionType.Exp,
                                     scale=-0.5)
                nc.sync.dma_start(out=r1e[0:1, :], in_=nmu_e)
                nc.sync.dma_start(out=r1e[1:2, :], in_=sd_e)
                a2ps = ea2.tile([128, S], F32)
                nc.tensor.matmul(a2ps, ones1, rs2, start=True, stop=True)
                nc.vector.tensor_copy(out=a2_sb, in_=a2ps)

            with tc.tile_pool(name="kv_ps" + sfx, bufs=2, space="PSUM") as kvp:
                for ji in range(JI):
                    kvps = kvp.tile([128, S], F32)
                    for ei in range(EI):
                        nc.tensor.matmul(kvps, wkvT[:, ei, ji * 128:(ji + 1) * 128],
                                         eTb[:, ei, :],
                                         start=(ei == 0), stop=False)
                    nc.tensor.matmul(kvps, wkvr[:, ji * 128:(ji + 1) * 128],
                                     r1e, start=False, stop=True)
                    nc.vector.tensor_mul(kvT[:, ji, :], kvps, a2_sb)

            with tc.tile_pool(name="v_tp" + sfx, bufs=2, space="PSUM") as vtp:
                for jj in range(CI):
                    for si in range(2):
                        tp = vtp.tile([128, 128], BF16, tag="vtp")
                        nc.tensor.transpose(tp, kvT[:, CI + jj, si * 128:(si + 1) * 128], idb)
                        nc.vector.tensor_copy(out=vnat[:, si, jj * 128:(jj + 1) * 128], in_=tp)

            # ---- decoder x: load, stats, q ----
            with tc.tile_pool(name="x_f32" + sfx, bufs=3) as xfp, \
                 tc.tile_pool(name="x_sq" + sfx, bufs=2) as xsqp, \
                 tc.tile_pool(name="x_row" + sfx, bufs=2, space="PSUM") as xrow, \
                 tc.tile_pool(name="x_a" + sfx, bufs=1, space="PSUM") as xa, \
                 tc.tile_pool(name="rows2" + sfx, bufs=3) as rows:

                sumx = xrow.tile([1, HW], F32, tag="xrow")
                sqx = xrow.tile([1, HW], F32, tag="xrow")
                for ci in range(CI):
                    xf = xfp.tile([128, HW], F32, tag="xf")
                    nc.sync.dma_start(
                        out=xf, in_=xd[b].rearrange("(ci p) hw -> p ci hw", p=128)[:, ci, :])
                    nc.gpsimd.tensor_copy(out=xsb[:, ci, :], in_=xf)
                    xq = xsqp.tile([128, HW], BF16, tag="xq")
                    nc.vector.tensor_mul(xq, xsb[:, ci, :], xsb[:, ci, :])
                    for ch in range(2):
                        sl = slice(ch * 512, (ch + 1) * 512)
                        nc.tensor.matmul(sumx[:, sl], ones_cb, xsb[:, ci, sl],
                                         start=(ci == 0), stop=(ci == CI - 1))
                        nc.tensor.matmul(sqx[:, sl], ones_cb, xq[:, sl],
                                         start=(ci == 0), stop=(ci == CI - 1))
                nc.scalar.activation(out=nmu_x, in_=sumx,
                                     func=mybir.ActivationFunctionType.Copy,
                                     scale=-1.0 / C)
                mu2 = rows.tile([1, HW], F32, tag="r_hw")
                nc.scalar.activation(out=mu2, in_=nmu_x,
                                     func=mybir.ActivationFunctionType.Square)
                ex2 = rows.tile([1, HW], F32, tag="r_hw")
                nc.scalar.activation(out=ex2, in_=sqx,
                                     func=mybir.ActivationFunctionType.Copy,
                                     scale=1.0 / C)
                var = rows.tile([1, HW], F32, tag="r_hw")
                nc.vector.tensor_tensor(out=var, in0=ex2, in1=mu2,
                                        op=mybir.AluOpType.subtract)
                lnv = rows.tile([1, HW], F32, tag="r_hw")
                nc.scalar.activation(out=lnv, in_=var,
                                     func=mybir.ActivationFunctionType.Ln,
                                     bias=eps11)
                nc.scalar.activation(out=sd_x, in_=lnv,
                                     func=mybir.ActivationFunctionType.Exp,
                                     scale=0.5)
                rsx = rows.tile([1, HW], F32, tag="r_hw")
                nc.scalar.activation(out=rsx, in_=lnv,
                                     func=mybir.ActivationFunctionType.Exp,
                                     scale=-0.5)
                nc.sync.dma_start(out=r1x[0:1, :], in_=nmu_x)
                nc.sync.dma_start(out=r1x[1:2, :], in_=sd_x)
                aps = xa.tile([128, HW], F32)
                for ch in range(2):
                    sl = slice(ch * 512, (ch + 1) * 512)
                    nc.tensor.matmul(aps[:, sl], ones1, rsx[:, sl], start=True, stop=True)
                nc.vector.tensor_copy(out=a_sb, in_=aps)

            with tc.tile_pool(name="q_ps" + sfx, bufs=2, space="PSUM") as qpp:
                for oi in range(CI):
                    qps = qpp.tile([128, HW], F32)
                    for ch in range(2):
                        sl = slice(ch * 512, (ch + 1) * 512)
                        for ci in range(CI):
                            nc.tensor.matmul(qps[:, sl],
                                             wqT[:, ci, oi * 128:(oi + 1) * 128],
                                             xsb[:, ci, sl],
                                             start=(ci == 0), stop=False)
                        nc.tensor.matmul(qps[:, sl],
                                         wqr[:, oi * 128:(oi + 1) * 128],
                                         r1x[:, sl], start=False, stop=True)
                    nc.vector.tensor_mul(qsb[:, oi, :], qps, a_sb)

            # ---- padding bias ----
            with tc.tile_pool(name="pad" + sfx, bufs=1) as padp:
                padi = padp.tile([128, 2], I32)
                nc.sync.dma_start(out=padi, in_=padd[b].rearrange("(si p) -> p si", p=128))
                padf = padp.tile([128, 2], F32)
                nc.vector.tensor_copy(out=padf, in_=padi)
                nc.scalar.mul(out=padneg, in_=padf, mul=-10000.0)

            # ---- attention ----
            # y accumulates UNNORMALIZED into ysb; per-(h,pc) softmax sums z
            # gather into zall rows (DVE copy), then ONE batched
            # 1/z = exp(-ln z) on ACT + 16 pair-broadcast matmuls + in-place
            # muls. Removes the per-iter 3.3us DVE iterative reciprocal.
            ysb = per.tile([128, CI, HW], BF16, tag="ysb")
            zall = per.tile([32, 512], F32, tag="zall")      # row = pc*16+h
            zln = per.tile([32, 512], F32, tag="zln")
            zinv = per.tile([32, 512], BF16, tag="zinv")
            zinv_t = per.tile([2, 16, 512], BF16, tag="zinv_t")
            with tc.tile_pool(name="s_ps" + sfx, bufs=3, space="PSUM") as spp, \
                 tc.tile_pool(name="z_ps" + sfx, bufs=2, space="PSUM") as zpp, \
                 tc.tile_pool(name="y_ps" + sfx, bufs=2, space="PSUM") as ypp, \
                 tc.tile_pool(name="att_sb" + sfx, bufs=3) as attp:
                for h in range(H):
                    ji = h // 2
                    dof = (h % 2) * 64
                    for pc in range(2):
                        psl = slice(pc * 512, (pc + 1) * 512)
                        eb = attp.tile([128, 2, 512], BF16, tag="eb")
                        for si in range(2):
                            sps = spp.tile([128, 512], F32, tag="sps")
                            nc.tensor.matmul(
                                sps,
                                kvT[dof:dof + 64, ji, si * 128:(si + 1) * 128],
                                qsb[dof:dof + 64, ji, psl],
                                start=True, stop=True)
                            nc.scalar.activation(out=eb[:, si, :], in_=sps,
                                                 func=mybir.ActivationFunctionType.Exp,
                                                 bias=padneg[:, si:si + 1],
                                                 scale=0.125)
                        zps = zpp.tile([1, 512], F32, tag="zps")
                        for si in range(2):
                            nc.tensor.matmul(zps, ones_cb, eb[:, si, :],
                                             start=(si == 0), stop=(si == 1))
                        # engines address SBUF at 32-partition granularity;
                        # stage at partition 0 then DMA to the gather row
                        zrow = attp.tile([1, 512], F32, tag="zrow")
                        nc.vector.tensor_copy(out=zrow, in_=zps)
                        nc.sync.dma_start(
                            out=zall[pc * 16 + h:pc * 16 + h + 1, :], in_=zrow)
                        yps = ypp.tile([64, 512], F32, tag="yps")
                        for si in range(2):
                            nc.tensor.matmul(yps, vnat[:, si, h * 64:(h + 1) * 64],
                                             eb[:, si, :],
                                             start=(si == 0), stop=(si == 1))
                        nc.vector.tensor_copy(out=ysb[dof:dof + 64, ji, psl],
                                              in_=yps)
            # normalize tail: 1/z, regroup head-pairs, broadcast, scale ysb
            nc.scalar.activation(out=zln, in_=zall,
                                 func=mybir.ActivationFunctionType.Ln)
            nc.scalar.activation(out=zinv, in_=zln,
                                 func=mybir.ActivationFunctionType.Exp,
                                 scale=-1.0)
            for pc in range(2):
                for jj in range(CI):
                    nc.sync.dma_start(
                        out=zinv_t[:, pc * 8 + jj, :],
                        in_=zinv[pc * 16 + 2 * jj:pc * 16 + 2 * jj + 2, :])
            with tc.tile_pool(name="zb_ps" + sfx, bufs=2, space="PSUM") as zbp:
                for pc in range(2):
                    psl = slice(pc * 512, (pc + 1) * 512)
                    for jj in range(CI):
                        zb = zbp.tile([128, 512], F32, tag="zb")
                        nc.tensor.matmul(zb, pmask, zinv_t[:, pc * 8 + jj, :],
                                         start=True, stop=True)
                        nc.vector.tensor_mul(ysb[:, jj, psl],
                                             ysb[:, jj, psl], zb)

            # ---- output: Wo @ y + bo + x ----
            with tc.tile_pool(name="o_ps" + sfx, bufs=2, space="PSUM") as opp, \
                 tc.tile_pool(name="o_sb" + sfx, bufs=2) as osp, \
                 tc.tile_pool(name="xr_sb" + sfx, bufs=2) as xrp:
                for oi in range(CI):
                    ops = opp.tile([128, HW], F32)
                    for ch in range(2):
                        sl = slice(ch * 512, (ch + 1) * 512)
                        for ci in range(CI):
                            nc.tensor.matmul(ops[:, sl],
                                             woT[:, ci, oi * 128:(oi + 1) * 128],
                                             ysb[:, ci, sl],
                                             start=(ci == 0), stop=(ci == CI - 1))

                    xres = xrp.tile([128, HW], F32, tag="xres")
                    nc.sync.dma_start(
                        out=xres, in_=xd[b].rearrange("(oi p) hw -> p oi hw", p=128)[:, oi, :])
                    # absorb the DMA wait on DVE so the residual add needs
                    # only the single PE wait (1 sync slot per DVE op)
                    nc.vector.tensor_copy(out=junk, in_=xres[0:1, 0:8])
                    osb = osp.tile([128, HW], F32, tag="osb")
                    nc.vector.scalar_tensor_tensor(
                        out=osb, in0=ops, scalar=bo_col[:, oi:oi + 1], in1=xres,
                        op0=mybir.AluOpType.add, op1=mybir.AluOpType.add)
                    nc.sync.dma_start(
                        out=outd[b].rearrange("(oi p) hw -> p oi hw", p=128)[:, oi, :],
                        in_=osb)
        per.release()
        scr.release()
        wgt.release()
        con.release()
    return nc


def _get_nc():
    if "nc" not in _CACHE:
        nc = bacc.Bacc()
        _build(nc)
        nc.compile()
        _CACHE["nc"] = nc
    return _CACHE["nc"]


def _prep_weights(gamma_dec, beta_dec, gamma_enc, beta_enc, Wq, bq, Wkv, bkv, Wo, bo):
    Wq = np.asarray(Wq, np.float32)
    Wkv = np.asarray(Wkv, np.float32)
    Wo = np.asarray(Wo, np.float32)
    gd = np.asarray(gamma_dec, np.float32)
    bd = np.asarray(beta_dec, np.float32)
    ge = np.asarray(gamma_enc, np.float32)
    be = np.asarray(beta_enc, np.float32)

    def packT(w):  # [o, c] -> [128, c//128, o] bf16 (stationary layout)
        o, c = w.shape
        t = np.ascontiguousarray(w.T.reshape(c // 128, 128, o).transpose(1, 0, 2))
        return t.astype(BF)

    wqg_full = Wq * gd[None, :]
    wqT = packT(wqg_full)
    wkvg_full = Wkv * ge[None, :]
    wkvT = packT(wkvg_full)
    woT = packT(Wo)
    # row sums from the bf16-rounded weights to match the device matmuls
    wqg = wqg_full.astype(BF).astype(np.float32).sum(axis=1)         # [C]
    wqb = Wq.astype(BF).astype(np.float32) @ bd + np.asarray(bq, np.float32)
    wkvg = wkvg_full.astype(BF).astype(np.float32).sum(axis=1)
    wkvb = Wkv.astype(BF).astype(np.float32) @ be + np.asarray(bkv, np.float32)
    wqr = np.ascontiguousarray(np.stack([wqg, wqb]).astype(BF))      # [2, C]
    wkvr = np.ascontiguousarray(np.stack([wkvg, wkvb]).astype(BF))
    pm = np.zeros((2, 128), np.float32)
    pm[0, :64] = 1.0
    pm[1, 64:] = 1.0
    return dict(
        wqT=wqT, wkvT=wkvT, woT=woT, wqr=wqr, wkvr=wkvr,
        bo=np.asarray(bo, np.float32), pmask=pm.astype(BF),
    )


def kernel(x, enc, padding, gamma_dec, beta_dec, gamma_enc, beta_enc,
           Wq, bq, Wkv, bkv, Wo, bo, _trace=False):
    nc = _get_nc()
    x = np.ascontiguousarray(np.asarray(x, np.float32)).reshape(B, C, HW)
    enc = np.ascontiguousarray(np.asarray(enc, np.float32))
    padding = np.ascontiguousarray(np.asarray(padding, np.int32))
    wdict = _prep_weights(gamma_dec, beta_dec, gamma_enc, beta_enc,
                          Wq, bq, Wkv, bkv, Wo, bo)
    in_maps = []
    for c in range(NCORES):
        m = dict(wdict)
        m["x"] = np.ascontiguousarray(x[c * BPC:(c + 1) * BPC])
        m["enc"] = np.ascontiguousarray(enc[c * BPC:(c + 1) * BPC])
        m["padding"] = np.ascontiguousarray(padding[c * BPC:(c + 1) * BPC])
        in_maps.append(m)
    res = run_bass_kernel_spmd(nc, in_maps, core_ids=list(range(NCORES)),
                               trace=_trace)
    if _trace:
        _CACHE["last_results"] = res
    out = np.concatenate([res.results[c]["out"] for c in range(NCORES)], axis=0)
    return out.reshape(B, C, 32, 32).astype(np.float32)

